# revision 42
# baseline (speedup 1.0000x reference)
"""GQA attention (B=2, S=2048, HID=2048, 32 q heads / 8 kv heads, fp32 I/O)
on 8 TRN2 NeuronCores.

Sharding: sequence-parallel with fully local K/V. Core c owns 512 query
tokens of batch c//4 (cores 0-3 = batch 0, cores 4-7 = batch 1), but
computes K^T and V for ALL 2048 tokens of its batch locally — that
(+~40% KV projection FLOPs) is much cheaper than an intra-chip
AllGather, which measures 100-170us and blockades the DMA engines while
it runs. Attention is permutation-invariant over keys, so each core
orders tokens own-block-first (host-side permutation) and the device
program stays rank-independent.

V carries a fused ones-column per kv head so the PV matmul also
produces the softmax row-sums; the output-projection bias is fused as
an extra contraction row. All matmuls run in bf16 with fp32 PSUM
accumulation (fp32 matmul is 4x slower on the PE). Heads are processed
in kv-parity pairs whose score matmuls occupy different PE row groups;
score PSUM tiles span two key chunks so each Exp covers N=1024,
amortizing ACT's fixed per-instruction overhead. The attention inner
loop is ScalarE(exp)-bound; Q-projection chunks are interleaved one
group ahead so they run in the PE's slack.

Softmax normalization: per head, 1/den via reciprocal_approx_fast on
the single den row (the PE's in-order queue must never wait on DVE —
the broadcast matmuls that consume the reciprocals are deferred by two
pairs, by which time the DVE work is long finished; the old exact
reciprocal cost 3.3us/head on one lane and stalled the PE every pair,
which also kept the PE HAM clock-gate oscillating at 1.2GHz).
Unnormalized head outputs are assembled into attnT pair tiles and
normalized there with a single [128,512] multiply per tile against a
K=1-matmul-broadcast reciprocal plane.

All transposes / casts / padding are done host-side in numpy.
"""

import functools
from contextlib import ExitStack

import numpy as np
import ml_dtypes

import concourse.bass as bass
import concourse.mybir as mybir
import concourse.tile as tile
from concourse import bacc
from concourse.bass_utils import run_bass_kernel_spmd

BF = mybir.dt.bfloat16
F32 = mybir.dt.float32

B, S, HID = 2, 2048, 2048
NH, NKV, HD = 32, 8, 64          # q heads, kv heads, head dim
GRP = NH // NKV                  # 4 q heads per kv head
TP = 4                           # cores per batch group
TOK = S // TP                    # 512 local query tokens per core
KC = HID // 128                  # 16 contraction chunks of 128
NKC = S // 128                   # 16 key chunks of 128 (full seq)
VW = NKV * (HD + 1)              # 520: V width incl. ones columns
EXP_SCALE = float(HD) ** -0.5    # 1/8 softmax scale, fused into Exp


def q_slot(h):
    """qTp tile index and partition base for head h.

    Head h lives at partition base ((h//4)%2)*64 — the same base its kv
    head kh=h//4 occupies inside the kTg tiles, so the scores matmul's
    lhsT and rhs stay partition-aligned. The other 64 partitions of its
    qTp tile are zero, so the score matmul runs with the full K=128
    contraction (the off-head kv rows multiply zeros): full-array
    matmuls keep Fast Weight Load eligible and LDWEIGHTS hidden, which
    measures ~100ns/matmul faster than K=64 row-tiled ones.
    """
    return ((h // 4) // 2) * 4 + (h % 4), ((h // 4) % 2) * 64


def build_graph():
    nc = bacc.Bacc(None, target_bir_lowering=False, debug=False, num_devices=8)

    xT = nc.declare_dram_parameter("xT", [HID, S], BF, isOutput=False)
    wkT = nc.declare_dram_parameter("wkT", [HID, NKV * HD], BF, isOutput=False)
    wvT = nc.declare_dram_parameter("wvT", [HID, NKV * HD], BF, isOutput=False)
    wqT = nc.declare_dram_parameter("wqT", [HID, HID], BF, isOutput=False)
    woT = nc.declare_dram_parameter("woT", [HID + 1, HID], BF, isOutput=False)
    selpD = nc.declare_dram_parameter("selpc", [8, 16 * 128], BF,
                                      isOutput=False)
    ohgD = nc.declare_dram_parameter("ohgc", [HD + 1, 64], BF, isOutput=False)
    out = nc.declare_dram_parameter("out", [TOK, HID], F32, isOutput=True)

    with tile.TileContext(nc) as tc, ExitStack() as es:
        pers = es.enter_context(tc.tile_pool(name="pers", bufs=1))

        def T(shape, dtype, *, name):
            return pers.tile(shape, dtype, name=name, tag=name)

        # Q-projection weight + staging pools outlive the xin pool, so they
        # must be opened first (pools close in stack order). The qTp tiles
        # also live below xin: allocating them in space that never aliases
        # xr/wk/wv means their zero-half fills and routing DMAs don't
        # inherit a write-after-read dependency on the whole of phase A.
        wqp = es.enter_context(tc.tile_pool(name="wqp", bufs=48))
        stgB = es.enter_context(tc.tile_pool(name="stgB", bufs=6))
        qtp = es.enter_context(tc.tile_pool(name="qtp", bufs=1))
        qTpA = [qtp.tile([128, TOK], BF, tag=f"qA{i}", name=f"qA{i}")
                for i in range(NH // 2)]
        qTpB = [qtp.tile([128, TOK], BF, tag=f"qB{i}", name=f"qB{i}")
                for i in range(NH // 2)]
        for i in range(NH // 2):
            nc.vector.memset(qTpA[i][64:128, :], 0.0)
            nc.vector.memset(qTpB[i][0:64, :], 0.0)

        # selp[:, t*128:(t+1)*128] is the K=8 broadcast-selection lhsT for
        # attnT tile t; ohg[64:65, r*8:(r+1)*8] is the K=1 one-hot lhsT
        # routing a den row (partition 64) to gden partition r. Both are
        # host-built constants (engine APs can't start at partitions 1-7,
        # and building them on-device costs ~30us of single-lane DVE).
        selp = T([8, 16 * 128], BF, name="selp")
        ohg = T([HD + 1, 64], BF, name="ohg")

        # ---- SBUF inputs; DMA issue order = priority --------------------
        # xq (own 512 query-token columns) + wk/wv first: they unblock
        # phase A's first key block; xr (the other 1536 tokens) streams in
        # behind and is only needed from key block nb=1 onward.
        xin_cm = tc.tile_pool(name="xin", bufs=1)
        xin = xin_cm.__enter__()
        xq = [T([128, TOK], BF, name=f"xq{k}") for k in range(KC)]
        xr = [xin.tile([128, S - TOK], BF, tag=f"xr{k}", name=f"xr{k}")
              for k in range(KC)]
        wk_sb = [xin.tile([128, NKV * HD], BF, tag=f"wk{k}", name=f"wk{k}")
                 for k in range(KC)]
        wv_sb = [xin.tile([128, NKV * HD], BF, tag=f"wv{k}", name=f"wv{k}")
                 for k in range(KC)]
        for k in range(KC):
            nc.sync.dma_start(out=xq[k][:, :], in_=xT[k * 128:(k + 1) * 128, 0:TOK])
            nc.sync.dma_start(out=wk_sb[k][:, :], in_=wkT[k * 128:(k + 1) * 128, :])
        for k in range(KC):
            nc.sync.dma_start(out=wv_sb[k][:, :], in_=wvT[k * 128:(k + 1) * 128, :])
        for k in range(KC):
            nc.sync.dma_start(out=xr[k][:, :], in_=xT[k * 128:(k + 1) * 128, TOK:S])
        nc.sync.dma_start(out=selp[:, :], in_=selpD[:, :])
        nc.sync.dma_start(out=ohg[:, :], in_=ohgD[:, :])

        def xcols(k, lo, n):
            # columns lo..lo+n of the permuted x^T chunk k
            return xq[k][:, lo:lo + n] if lo < TOK \
                else xr[k][:, lo - TOK:lo - TOK + n]

        ones128 = T([1, 128], BF, name="ones128")
        nc.vector.memset(ones128[:, :], 1.0)

        def gath_row(h):
            return 2 * (h % 4) + (h // 4) % 2

        # kTg[nb*4+mt]: [128, 512] = K^T rows mt*128.. for key block nb
        # (kv heads 2mt at partitions 0-63, 2mt+1 at 64-127).
        # vg[c]: [128, 1024] V_aug rows for key chunk c: kv head kh owns
        # cols kh*128..kh*128+127 as [V (64) | ones (1) | 1.0 pad (63)],
        # so the PV lhsT is a full 128-col weight tile (FWL-eligible, LDW
        # hidden); po rows 65:128 receive den copies and are never read.
        kTg = [T([128, TOK], BF, name=f"kTg{i}") for i in range(16)]
        vg = [T([128, NKV * 128], BF, name=f"vg{c}") for c in range(NKC)]
        # the ones/pad columns of vg are constant: set whole tiles to 1.0
        # once during the input-DMA wait (the V copies overwrite the V
        # columns); keeps the per-chunk evacuation to pure copies
        for c in range(NKC):
            nc.vector.memset(vg[c][:, :], 1.0)
        # pre-load the ACT exp table set while the PE is still in phase A
        dmy = T([1, 32], F32, name="dmy")
        nc.vector.memset(dmy[:, :], 0.0)
        nc.scalar.activation(out=dmy[:, :], in_=dmy[:, :],
                             func=mybir.ActivationFunctionType.Exp, scale=1.0)

        # =============== phase A: K^T and V_aug for the whole batch ======
        # group-0 Q chunks are computed at the start of key block nb=3 so
        # their qTp routing DMAs land during phase A — the first score
        # matmuls then start right at phase A's end instead of waiting out
        # a Q-projection + routing chain.
        def emit_q_chunk(m, psum_pool, tag):
            ps = psum_pool.tile([128, TOK], F32, tag=tag, name=f"psq{m}")
            for k in range(KC):
                w = wqp.tile([128, 128], BF, tag="wq", name=f"wq{m}_{k}")
                nc.sync.dma_start(
                    out=w[:, :],
                    in_=wqT[k * 128:(k + 1) * 128, m * 128:(m + 1) * 128])
                nc.tensor.matmul(
                    out=ps[:, :], lhsT=w[:, :], rhs=xq[k][:, :],
                    start=(k == 0), stop=(k == KC - 1))
            st = stgB.tile([128, TOK], BF, tag="stg", name=f"stq{m}")
            nc.vector.tensor_copy(out=st[:, :], in_=ps[:, :])
            # route each head to its kv-parity-aligned slot via DMA
            for j in range(2):
                h = 2 * m + j
                i, roff = q_slot(h)
                dst = qTpA[i] if roff == 0 else qTpB[i]
                nc.sync.dma_start(out=dst[roff:roff + 64, :],
                                  in_=st[j * 64:(j + 1) * 64, :])

        with tc.tile_pool(name="accA", bufs=4, space="PSUM") as accA:
            for nb in range(TP):
                if nb == 3:
                    for m in (0, 2, 1, 3):
                        emit_q_chunk(m, accA, "acc")
                for mt in range(NKV // 2):
                    ps = accA.tile([128, TOK], F32, tag="acc",
                                   name=f"psk{nb}_{mt}")
                    for k in range(KC):
                        nc.tensor.matmul(
                            out=ps[:, :],
                            lhsT=wk_sb[k][:, mt * 128:(mt + 1) * 128],
                            rhs=xcols(k, nb * TOK, TOK),
                            start=(k == 0), stop=(k == KC - 1))
                    nc.vector.tensor_copy(out=kTg[nb * 4 + mt][:, :],
                                          in_=ps[:, :])
                for tc4 in range(TP):
                    c = nb * 4 + tc4
                    ps = accA.tile([128, NKV * HD], F32, tag="acc",
                                   name=f"psv{c}")
                    for k in range(KC):
                        nc.tensor.matmul(
                            out=ps[:, :],
                            lhsT=xcols(k, c * 128, 128),
                            rhs=wv_sb[k][:, :],
                            start=(k == 0), stop=(k == KC - 1))
                    for kh in range(NKV):
                        nc.vector.tensor_copy(
                            out=vg[c][:, kh * 128:kh * 128 + HD],
                            in_=ps[:, kh * HD:(kh + 1) * HD])
        xin_cm.__exit__(None, None, None)

        # attention-persistent tiles reuse the space freed by xr/wk/wv:
        # per-head zero-padded Q tiles (side-A heads at partitions 0:64 of
        # qTpA, side-B at 64:128 of qTpB, the other half zero) and the
        # unnormalized attention-output pair tiles.
        # unnormalized attention-output pair tiles reuse the space freed by
        # xr/wk/wv (first written a full pair into attention, long after
        # the last xr readers retire)
        apers = es.enter_context(tc.tile_pool(name="apers", bufs=1))
        attnT = [apers.tile([128, TOK], BF, tag=f"at{t}", name=f"at{t}")
                 for t in range(NH // 2)]

        # =============== phases B+D interleaved: Q chunks + attention ====
        # Q chunk m (q dims m*128..) fills heads 2m, 2m+1. Pair group qg
        # (pairs 4qg..4qg+3) needs exactly Q chunks {4qg, 4qg+2, 4qg+1,
        # 4qg+3}. Group 0's were computed inside phase A; group g's are
        # spread one per pair across group g-1 so the DMA+matmul latency
        # hides in the attention slack.
        pairs = []
        for g in range(0, NKV, 2):
            for j in range(GRP):
                pairs.append((g * GRP + j, (g + 1) * GRP + j))

        # chunks for group g in emission order
        def group_chunks(g):
            return [4 * g, 4 * g + 2, 4 * g + 1, 4 * g + 3]

        # attnT tile t is normalized with one K=8 broadcast matmul (den
        # reciprocals of both heads onto their partition halves) + one
        # in-place [128,512] multiply. rcb_of_group[g] is the group's
        # gathered bf16 reciprocal tile.
        rcb_of_group = {}

        def emit_norm_tile(t, scr):
            pb = scr.tile([128, TOK], F32, tag="scr", name=f"pb{t}")
            nc.tensor.matmul(
                out=pb[:, :], lhsT=selp[:, t * 128:(t + 1) * 128],
                rhs=rcb_of_group[t // 4][0:8, :], start=True, stop=True)
            nc.vector.tensor_mul(out=attnT[t][:, :], in0=attnT[t][:, :],
                                 in1=pb[:, :])

        # wop opened early so Wo tiles prefetch during attention; rcp
        # lives on the ExitStack because the last group's reciprocals are
        # consumed inside phase E
        wop = es.enter_context(tc.tile_pool(name="wop", bufs=24))
        rcp = es.enter_context(tc.tile_pool(name="rcp", bufs=2))
        with tc.tile_pool(name="scr", bufs=1, space="PSUM") as scr, \
             tc.tile_pool(name="sps", bufs=2, space="PSUM") as sps, \
             tc.tile_pool(name="ops", bufs=2, space="PSUM") as ops, \
             tc.tile_pool(name="gdp", bufs=1, space="PSUM") as gdp, \
             tc.tile_pool(name="ptp", bufs=4) as ptp, \
             tc.tile_pool(name="sdp", bufs=2) as sdp, \
             tc.tile_pool(name="tmp", bufs=2) as tmp:
            gden = None
            for pi, (hA, hB) in enumerate(pairs):
                # deferred normalize: tile k at pair k+5, by which point the
                # group's batched reciprocal is long finished
                if pi >= 5:
                    emit_norm_tile(pi - 5, scr)
                if pi % 4 == 0:
                    gden = gdp.tile([8, TOK], F32, tag="gd",
                                    name=f"gd{pi // 4}")
                g = pi // 4 + 1
                if g <= 3:
                    emit_q_chunk(group_chunks(g)[pi % 4], scr, "scr")
                khA, khB = hA // GRP, hB // GRP
                kt = khA // 2      # kTg row tile: khA at 0:64, khB at 64:128
                qiA, _ = q_slot(hA)
                qiB, _ = q_slot(hB)
                poA = ops.tile([128, TOK], F32, tag="po", name=f"poA{hA}")
                poB = ops.tile([128, TOK], F32, tag="po", name=f"poB{hB}")
                pts = []
                for ci in range(0, NKC, 2):
                    psA = sps.tile([128, 2 * TOK], F32, tag="ps",
                                   name=f"psA{hA}_{ci}")
                    psB = sps.tile([128, 2 * TOK], F32, tag="ps",
                                   name=f"psB{hB}_{ci}")
                    for dc in range(2):
                        c = ci + dc
                        nb, lc = c // 4, c % 4
                        kts = kTg[nb * 4 + kt]
                        nc.tensor.matmul(
                            out=psA[:, dc * TOK:(dc + 1) * TOK],
                            lhsT=kts[:, lc * 128:(lc + 1) * 128],
                            rhs=qTpA[qiA][:, :], start=True, stop=True)
                        nc.tensor.matmul(
                            out=psB[:, dc * TOK:(dc + 1) * TOK],
                            lhsT=kts[:, lc * 128:(lc + 1) * 128],
                            rhs=qTpB[qiB][:, :], start=True, stop=True)
                    ptA = ptp.tile([128, 2 * TOK], BF, tag="pt",
                                   name=f"ptA{hA}_{ci}")
                    nc.scalar.activation(
                        out=ptA[:, :], in_=psA[:, :],
                        func=mybir.ActivationFunctionType.Exp, scale=EXP_SCALE)
                    ptB = ptp.tile([128, 2 * TOK], BF, tag="pt",
                                   name=f"ptB{hB}_{ci}")
                    nc.scalar.activation(
                        out=ptB[:, :], in_=psB[:, :],
                        func=mybir.ActivationFunctionType.Exp, scale=EXP_SCALE)
                    pts.append((ci, ptA, ptB))
                for ci, ptA, ptB in pts:
                    for dc in range(2):
                        c = ci + dc
                        nc.tensor.matmul(
                            out=poA[:, :],
                            lhsT=vg[c][:, khA * 128:(khA + 1) * 128],
                            rhs=ptA[:, dc * TOK:(dc + 1) * TOK],
                            start=(c == 0), stop=(c == NKC - 1))
                        nc.tensor.matmul(
                            out=poB[:, :],
                            lhsT=vg[c][:, khB * 128:(khB + 1) * 128],
                            rhs=ptB[:, dc * TOK:(dc + 1) * TOK],
                            start=(c == 0), stop=(c == NKC - 1))
                # per-head: den row PSUM->SBUF (DVE, bf16), then a K=1
                # one-hot matmul routes it onto gden partition r (engine
                # APs cannot start at arbitrary partitions; matmul output
                # at partition base 0 can). Unnormalized numerator rows go
                # into the attnT pair tiles (direct DVE cast for even
                # heads, DMA partition-shift for odd heads).
                sd = sdp.tile([HD + 1, 2 * TOK], BF, tag="sd",
                              name=f"sd{pi}")
                j = pi % 4
                for h, po, cb in ((hA, poA, 0), (hB, poB, TOK)):
                    nc.vector.tensor_copy(out=sd[HD:HD + 1, cb:cb + TOK],
                                          in_=po[HD:HD + 1, :])
                    r = gath_row(h)
                    nc.tensor.matmul(
                        out=gden[:, :], lhsT=ohg[HD:HD + 1, 8 * r:8 * r + 8],
                        rhs=sd[HD:HD + 1, cb:cb + TOK],
                        start=(j == 0 and cb == 0), stop=(j == 3 and cb != 0))
                    t, half = h // 2, (h % 2) * 64
                    if half == 0:
                        nc.vector.tensor_copy(out=attnT[t][0:64, :],
                                              in_=po[0:HD, :])
                    else:
                        tb = tmp.tile([64, TOK], BF, tag="tb", name=f"tb{h}")
                        nc.vector.tensor_copy(out=tb[:, :], in_=po[0:HD, :])
                        nc.sync.dma_start(out=attnT[t][64:128, :],
                                          in_=tb[:, :])
                if pi % 4 == 3:
                    # group complete: one exact batched reciprocal for all
                    # 8 den rows + bf16 cast
                    g = pi // 4
                    rc = rcp.tile([8, TOK], F32, tag="rc", name=f"rc{g}")
                    nc.vector.reciprocal(out=rc[:, :], in_=gden[:, :])
                    rcb = rcp.tile([8, TOK], BF, tag="rcb", name=f"rcb{g}")
                    nc.vector.tensor_copy(out=rcb[:, :], in_=rc[:, :])
                    rcb_of_group[g] = rcb
        # =============== phase E: output projection + bias ===========
        # the last group's normalize tiles (11-15) are emitted after nt=0's
        # first 11 contraction steps: those matmuls only read attnT[0..10],
        # so the PE crunches them while the group-3 reciprocal finishes on
        # DVE instead of idling on it
        with tc.tile_pool(name="yps", bufs=4, space="PSUM") as yps, \
             tc.tile_pool(name="pbE", bufs=2, space="PSUM") as pbE, \
             tc.tile_pool(name="ystg", bufs=3) as ystg:
            for nt in range(4):        # 4 output column blocks of 512
                wo_last = wop.tile([1, 512], BF, tag="wolast",
                                   name=f"wl{nt}")
                nc.sync.dma_start(
                    out=wo_last[:, :],
                    in_=woT[HID:HID + 1, nt * 512:(nt + 1) * 512])
                pys = [yps.tile([128, 512], F32, tag="py",
                                name=f"py{nt}_{i}") for i in range(4)]
                for kc in range(KC):
                    if nt == 0 and kc == 11:
                        for t in range(11, 16):
                            emit_norm_tile(t, pbE)
                    wo_t = wop.tile([128, 512], BF, tag="wo",
                                    name=f"wo{nt}_{kc}")
                    nc.sync.dma_start(
                        out=wo_t[:, :],
                        in_=woT[kc * 128:(kc + 1) * 128,
                                nt * 512:(nt + 1) * 512])
                    for mt in range(4):
                        nc.tensor.matmul(
                            out=pys[mt][:, :],
                            lhsT=attnT[kc][:, mt * 128:(mt + 1) * 128],
                            rhs=wo_t[:, :],
                            start=(kc == 0), stop=False)
                for mt in range(4):    # bias via ones row, K=1 matmul
                    nc.tensor.matmul(
                        out=pys[mt][:, :], lhsT=ones128[:, :],
                        rhs=wo_last[:, :], start=False, stop=True)
                    ys = ystg.tile([128, 512], F32, tag="ys",
                                   name=f"ys{nt}_{mt}")
                    nc.vector.tensor_copy(out=ys[:, :], in_=pys[mt][:, :])
                    nc.sync.dma_start(
                        out=out[mt * 128:(mt + 1) * 128,
                                nt * 512:(nt + 1) * 512],
                        in_=ys[:, :])

    nc.finalize()
    return nc


@functools.lru_cache(maxsize=1)
def _graph():
    return build_graph()


def make_in_maps(x, Wq, Wk, Wv, Wo, bo):
    bf16 = ml_dtypes.bfloat16
    x = np.asarray(x, np.float32)
    wqT = np.ascontiguousarray(np.asarray(Wq, np.float32).T).astype(bf16)
    wkT = np.ascontiguousarray(np.asarray(Wk, np.float32).T).astype(bf16)
    wvT = np.ascontiguousarray(np.asarray(Wv, np.float32).T).astype(bf16)
    woT = np.concatenate(
        [np.asarray(Wo, np.float32).T,
         np.asarray(bo, np.float32)[None, :]], axis=0).astype(bf16)
    woT = np.ascontiguousarray(woT)

    # device-side selection constants (see build_graph)
    def gath_row(h):
        return 2 * (h % 4) + (h // 4) % 2

    selp = np.zeros((8, 16 * 128), np.float32)
    for t in range(16):
        for half in range(2):
            r = gath_row(2 * t + half)
            selp[r, t * 128 + half * 64:t * 128 + half * 64 + 64] = 1.0
    selp = selp.astype(bf16)
    ohg = np.zeros((65, 64), np.float32)
    for r in range(8):
        ohg[64, 9 * r] = 1.0
    ohg = ohg.astype(bf16)

    in_maps = []
    for c in range(8):
        b, r = c // TP, c % TP
        # token permutation: own query block first, rest after (attention
        # is permutation-invariant over keys)
        perm = np.r_[r * TOK:(r + 1) * TOK, 0:r * TOK, (r + 1) * TOK:S]
        xT_c = np.ascontiguousarray(x[b].T[:, perm]).astype(bf16)
        in_maps.append(
            {"xT": xT_c, "wqT": wqT, "wkT": wkT, "wvT": wvT, "woT": woT,
             "selpc": selp, "ohgc": ohg})
    return in_maps


def kernel(x, Wq, Wk, Wv, Wo, bo):
    nc = _graph()
    in_maps = make_in_maps(x, Wq, Wk, Wv, Wo, bo)
    res = run_bass_kernel_spmd(nc, in_maps, core_ids=list(range(8)))
    out = np.empty((B, S, HID), np.float32)
    for c in range(8):
        b, r = c // TP, c % TP
        out[b, r * TOK:(r + 1) * TOK, :] = np.asarray(
            res.results[c]["out"], np.float32)
    return out
